# revision 34
# baseline (speedup 1.0000x reference)
"""Trainium2 Bass kernel: transformer encoder layer (B=4, S=2048, D=1024, H=16, FF=4096).

Sharding (8 NeuronCores, no collectives): core c handles batch b=c//2 and
query-token half r=c%2 (1024 query rows). K/V are recomputed per core over the
batch's full 2048-token sequence. All activations feature-on-partition.

fp8 everywhere the error budget allows (tolerance 2e-2; this lands ~6e-3):
- QKV/w_o projections run fp8-e4m3 DoubleRow (contraction-chunk pairs
  interleaved in the free dim -> 2x PE rate). Host pre-scales by powers of 2
  (x*16, w*64); epilogues rescale via ACT scale / DVE tensor_scalar.
- Softmax probs are fp8-e5m2: ScalarE writes Exp directly to fp8; VectorE uses
  an int8 Schraudolph (round(s*A+B) bitcast e5m2). A/V matmul is fp8 DoubleRow
  over kv-chunk pairs; V is e4m3*8. The softmax denominator rides as a 65th V
  column = 0.25, so 1/den is directly the ctx normalizer at ctx scale 32.
- Attention runs 512-wide query chunks; the per-kc scores PSUM tile packs both
  heads of a pair as free-dim planes so one [P,1024] exp op covers both, with
  exp work split ~50/50 between ScalarE and VectorE.
- FFN stays bf16 (fp8 there would blow the error budget) in two 512-wide
  groups; each group's FFN is a generator whose PE slices are drained one per
  attention unit, hiding exp latency under FFN matmuls. Residual stream is
  bf16 (hA: wo+residual -> LN1; h1b: LN1 out; hB: fc2+residual -> LN2) with
  separate tiles per role so group boundaries carry no WAR stalls.
- b_o + b_v@w_o is folded into the DMA'd residual (xtb = x + bo); b_q/b_k/b1/
  b2/be1 and g1 are folded into biases/weights on the host.
"""

import numpy as np
import ml_dtypes

import concourse.bass as bass
import concourse.tile as tile
from concourse import bacc
from concourse import mybir
from concourse.bass_utils import run_bass_kernel_spmd

P = 128
D = 1024          # d_model
S = 2048          # kv sequence length per core (one full batch)
TQ = 1024         # query tokens per core
H = 16            # heads
DK = 64           # head dim
FF = 4096         # ffn dim
DO = D // P       # 8  d_model chunks
KC = S // P       # 16 kv-token chunks
FO = FF // P      # 32 ffn chunks
NF = 512          # attention query-chunk width
NQA = TQ // NF    # 2 attention chunks
NG = 512          # ffn group width (2 attention chunks)
NW = 512          # projection free-dim tile
KH = KC // 2      # kv chunks per half
KP = KH // 2      # kv chunk-pairs per half

# fp8 power-of-2 scales
SX, SW, SQ, SV, SCTX = 16.0, 64.0, 2.0, 8.0, 32.0
KQ_SCALE = SQ / (SX * SW)        # 2^-9 on the QKV psum
V_SCALE = SV / (SX * SW)         # 2^-7
WO_SCALE = 1.0 / (SCTX * SW)     # 2^-11
EXP_SCALE = 1.0 / (8.0 * SQ * SQ)  # scores psum = 4*qk; exp arg = qk/8

# Schraudolph exp-to-e5m2: bits8 = round(s*A + B); exp(s*EXP_SCALE) ~ bitcast
SCH5_A = 4.0 * 1.4426950408889634 * EXP_SCALE   # 2^2 mantissa steps / octave
SCH5_B = 15.0 * 4.0 - 0.2316                    # exp-bias*4 - schraudolph corr

BF16 = mybir.dt.bfloat16
F32 = mybir.dt.float32
F32R = mybir.dt.float32r
F8E4 = mybir.dt.float8e4
F8E5 = mybir.dt.float8e5
I8 = mybir.dt.int8
AF = mybir.ActivationFunctionType
ALU = mybir.AluOpType
DR = mybir.MatmulPerfMode.DoubleRow


def build():
    nc = bacc.Bacc("TRN2", target_bir_lowering=False, debug=False, num_devices=8)

    xt8 = nc.dram_tensor("xt8", [P, 4, 2, S], F8E4, kind="ExternalInput").ap()
    wq8 = nc.dram_tensor("wq8", [P, 4, 2, D], F8E4, kind="ExternalInput").ap()
    wk8 = nc.dram_tensor("wk8", [P, 4, 2, D], F8E4, kind="ExternalInput").ap()
    wv8 = nc.dram_tensor("wv8", [P, 4, 2, D], F8E4, kind="ExternalInput").ap()
    wo8 = nc.dram_tensor("wo8", [P, 4, 2, D], F8E4, kind="ExternalInput").ap()
    xtb = nc.dram_tensor("xtb", [D, TQ], BF16, kind="ExternalInput").ap()
    w1s = nc.dram_tensor("w1s", [8, P, DO, 512], BF16, kind="ExternalInput").ap()
    w2s = nc.dram_tensor("w2s", [8, P, FO, P], BF16, kind="ExternalInput").ap()
    bq2 = nc.dram_tensor("bq2", [D], F32, kind="ExternalInput").ap()   # 2*b_q
    bk2 = nc.dram_tensor("bk2", [D], F32, kind="ExternalInput").ap()   # 2*b_k
    b1p = nc.dram_tensor("b1p", [FF], F32, kind="ExternalInput").ap()  # b1 + be1@w1
    b2e = nc.dram_tensor("b2e", [D], F32, kind="ExternalInput").ap()   # b2 + be1
    g1 = nc.dram_tensor("g1", [D], F32, kind="ExternalInput").ap()
    g2 = nc.dram_tensor("g2", [D], F32, kind="ExternalInput").ap()
    be2 = nc.dram_tensor("be2", [D], F32, kind="ExternalInput").ap()
    onesr = nc.dram_tensor("onesr", [P], BF16, kind="ExternalInput").ap()
    yt = nc.dram_tensor("yt", [D, TQ], F32R, kind="ExternalOutput").ap()

    xtb3 = xtb.rearrange("(o p) t -> p o t", p=P)
    yt3 = yt.rearrange("(o p) t -> p o t", p=P)

    with tile.TileContext(nc) as tc:
        with (
            tc.tile_pool(name="persist", bufs=1) as persist,
            tc.tile_pool(name="lns", bufs=1) as lns,
            tc.tile_pool(name="recp", bufs=2) as recp,
            tc.tile_pool(name="work", bufs=2) as work,
            tc.tile_pool(name="bcp", bufs=2) as bcp,
            tc.tile_pool(name="mm", bufs=2, space="PSUM") as mmp,
            tc.tile_pool(name="pssc", bufs=2, space="PSUM") as pssc,
            tc.tile_pool(name="pcp", bufs=2, space="PSUM") as pcp,
        ):
            def load_vec(ap, n_chunks, name):
                t = persist.tile([P, n_chunks], F32, tag=name)
                nc.sync.dma_start(out=t[:], in_=ap.rearrange("(o p) -> p o", p=P))
                return t

            eng_flip = [0]

            with tc.tile_pool(name="kqv", bufs=1) as kqvp:
                kT = kqvp.tile([P, DO, S], F8E4, tag="kT")
                qT = kqvp.tile([P, DO, TQ], F8E4, tag="qT")
                vaug = kqvp.tile([P, KC // 2, 2, H * 65], F8E4, tag="vaug")
                vaug_h = vaug.rearrange("p g l (h w) -> p g l h w", w=65)

                # ---------------- phase 1: QKV projections (fp8 DoubleRow) ----
                with (
                    tc.tile_pool(name="xtp", bufs=1) as xtp,
                    tc.tile_pool(name="wst", bufs=2) as wst,
                ):
                    wk_sb = wst.tile([P, 4, 2, D], F8E4, tag="w", name="w_k")
                    nc.sync.dma_start(out=wk_sb[:, 0:2], in_=wk8[:, 0:2])
                    nc.sync.dma_start(out=wk_sb[:, 2:4], in_=wk8[:, 2:4])
                    xt_sb = xtp.tile([P, 4, 2, S], F8E4, tag="xt")
                    for n in range(S // NW):
                        nc.sync.dma_start(out=xt_sb[:, :, :, bass.ts(n, NW)],
                                          in_=xt8[:, :, :, bass.ts(n, NW)])
                    bq_sb = load_vec(bq2, DO, "bq")
                    bk_sb = load_vec(bk2, DO, "bk")
                    b2e_sb = load_vec(b2e, DO, "b2e")
                    g1_sb = load_vec(g1, DO, "g1")
                    g2_sb = load_vec(g2, DO, "g2")
                    be2_sb = load_vec(be2, DO, "be2")
                    b1p_sb = load_vec(b1p, FO, "b1p")
                    ones_col = persist.tile([P, 1], BF16, tag="ones_col")
                    nc.sync.dma_start(out=ones_col[:], in_=onesr[:, None])
                    for t in range(KC):
                        nc.vector.memset(vaug_h[:, t // 2, t % 2, :, 64:65], 0.25)

                    def proj_epilogue(out_ap, ps, bias_col, scale):
                        # alternate ACT/DVE so both engines share QKV epilogues
                        eng_flip[0] ^= 1
                        if eng_flip[0]:
                            if bias_col is None:
                                nc.scalar.activation(out=out_ap, in_=ps,
                                                     func=AF.Identity, scale=scale)
                            else:
                                nc.scalar.activation(out=out_ap, in_=ps,
                                                     func=AF.Identity,
                                                     bias=bias_col, scale=scale)
                        else:
                            if bias_col is None:
                                nc.vector.tensor_scalar(
                                    out=out_ap, in0=ps, scalar1=scale,
                                    scalar2=None, op0=ALU.mult)
                            else:
                                nc.vector.tensor_scalar(
                                    out=out_ap, in0=ps, scalar1=scale,
                                    scalar2=bias_col, op0=ALU.mult, op1=ALU.add)

                    def kq_slice(w_sb, out_sb, bias_sb, n, m, nm):
                        ps = mmp.tile([P, NW], F32, tag="mm", name=f"mm_{nm}")
                        for kp in range(4):
                            nc.tensor.matmul(
                                ps[:], lhsT=w_sb[:, kp, :, bass.ts(m, P)],
                                rhs=xt_sb[:, kp, :, bass.ts(n, NW)],
                                start=(kp == 0), stop=(kp == 3),
                                perf_mode=DR)
                        proj_epilogue(out_sb[:, m, bass.ts(n, NW)], ps[:],
                                      bias_sb[:, m:m + 1], KQ_SCALE)

                    for n in range(S // NW):
                        for m in range(DO):
                            kq_slice(wk_sb, kT, bk_sb, n, m, f"k{n}_{m}")
                    wq_sb = wst.tile([P, 4, 2, D], F8E4, tag="w", name="w_q")
                    nc.sync.dma_start(out=wq_sb[:], in_=wq8)
                    for n in range(TQ // NW):
                        for m in range(DO):
                            kq_slice(wq_sb, qT, bq_sb, n, m, f"q{n}_{m}")

                    # V in natural [token, d] layout, heads padded to 65 cols
                    wv_sb = wst.tile([P, 4, 2, D], F8E4, tag="w", name="w_v")
                    nc.sync.dma_start(out=wv_sb[:], in_=wv8)
                    for t in range(KC):
                        for n in range(2):
                            ps = mmp.tile([P, NW], F32, tag="mm",
                                          name=f"mmv_{t}_{n}")
                            for kp in range(4):
                                nc.tensor.matmul(
                                    ps[:], lhsT=xt_sb[:, kp, :, bass.ts(t, P)],
                                    rhs=wv_sb[:, kp, :, bass.ts(n, NW)],
                                    start=(kp == 0), stop=(kp == 3),
                                    perf_mode=DR)
                            psh = ps.rearrange("p (h w) -> p h w", w=DK)
                            dst = vaug_h[:, t // 2, t % 2, 8 * n:8 * n + 8, 0:64]
                            proj_epilogue(dst, psh, None, V_SCALE)

                # ------------- phase 2: attention + interleaved FFN -----------
                with (
                    tc.tile_pool(name="es", bufs=2) as esp,
                    tc.tile_pool(name="ctxp", bufs=2) as ctxp,
                    tc.tile_pool(name="wop", bufs=1) as wop,
                    tc.tile_pool(name="w12", bufs=2) as w12,
                    tc.tile_pool(name="ffs2", bufs=1) as ffs2,
                    tc.tile_pool(name="ffh1", bufs=1) as ffh1,
                    tc.tile_pool(name="ffhb", bufs=1) as ffhb,
                    tc.tile_pool(name="ffa", bufs=1) as ffa,
                ):
                    wo_sb = wop.tile([P, 4, 2, D], F8E4, tag="wo")
                    nc.sync.dma_start(out=wo_sb[:], in_=wo8)

                    exp_acc = [0.0]
                    ACT_FRAC = 0.58

                    def emit_exp(dst_f8, ps):
                        """softmax exp of one [P,2,NF] scores tile -> e5m2."""
                        exp_acc[0] += ACT_FRAC
                        if exp_acc[0] >= 1.0:
                            exp_acc[0] -= 1.0
                            nc.scalar.activation(out=dst_f8, in_=ps[:],
                                                 func=AF.Exp, scale=EXP_SCALE)
                        else:
                            nc.vector.tensor_scalar(
                                out=dst_f8.bitcast(I8), in0=ps[:],
                                scalar1=SCH5_A, scalar2=SCH5_B,
                                op0=ALU.mult, op1=ALU.add)

                    def emit_scores(qn, j, half):
                        """scores+exp for 8 kv chunks x both heads of pair j.
                        es layout [P, idx, KH, NF]; psum tiles pack idx0/idx1
                        as free-dim planes so one [P,1024] exp covers both."""
                        qsl = bass.ts(qn, NF)
                        es_u = esp.tile([P, 2, KH, NF], F8E5, tag="es",
                                        name=f"es_{qn}_{j}_{half}")
                        for kl in range(KH):
                            kc = half * KH + kl
                            pss = pssc.tile([P, 2, NF], F32, tag="sc",
                                            name=f"sc_{kl % 2}")
                            for idx in range(2):
                                off = idx * DK
                                nc.tensor.matmul(
                                    pss[:, idx, :],
                                    lhsT=kT[off:off + DK, j, bass.ts(kc, P)],
                                    rhs=qT[off:off + DK, j, qsl],
                                    start=True, stop=True)
                            emit_exp(es_u[:, :, kl, :], pss)
                        return es_u

                    def emit_av(qn, j, half, es_u, pcs, ctx_t):
                        for idx in range(2):
                            h = 2 * j + idx
                            pc = pcs[idx]
                            for kp in range(KP):
                                pg = half * KP + kp
                                nc.tensor.matmul(
                                    pc[0:DK + 1, :],
                                    lhsT=vaug[:, pg, :, h * 65:(h + 1) * 65],
                                    rhs=es_u[:, idx, 2 * kp:2 * kp + 2, :],
                                    start=(pg == 0), stop=(pg == 2 * KP - 1),
                                    perf_mode=DR)
                            if half == 0:
                                continue
                            den = lns.tile([1, NF], F32, tag="den",
                                           name=f"den_{qn}_{h}")
                            nc.vector.tensor_copy(den[:], pc[DK:DK + 1, :])
                            rec = recp.tile([1, NF], F32, tag="rec",
                                            name=f"rec_{qn}_{h}")
                            nc.vector.reciprocal_approx_fast(
                                out=rec[:], in_=den[:])
                            recb = bcp.tile([DK, NF], F32, tag="recb",
                                            name=f"recb_{qn}_{h}")
                            nc.gpsimd.partition_broadcast(recb[:], rec[:],
                                                          channels=DK)
                            nc.vector.tensor_mul(
                                ctx_t[idx * DK:(idx + 1) * DK, j, :],
                                pc[0:DK, :], recb[:])

                    # ---- FFN for one 512-wide group, as PE-slice generator ----
                    def ln_stats(h3, tagp):
                        """LN over the partition (feature) dim of h3 [P,DO,NG].
                        sum in psum row 0, sumsq in row 32 (col-packed).
                        Yields once mid-way; returns (rstd_b, negms_b)."""
                        ps_s = mmp.tile([P, NW], F32, tag="mm", name=f"{tagp}_s")
                        ps_q = mmp.tile([P, NW], F32, tag="mm", name=f"{tagp}_q")
                        for o in range(DO):
                            nc.tensor.matmul(
                                ps_s[0:1, 0:NG], lhsT=ones_col,
                                rhs=h3[:, o, :],
                                start=(o == 0), stop=(o == DO - 1))
                            sq = work.tile([P, NG], BF16, tag="sq",
                                           name=f"{tagp}_sq{o % 2}")
                            nc.scalar.activation(out=sq[:], in_=h3[:, o, :],
                                                 func=AF.Square)
                            nc.tensor.matmul(
                                ps_q[0:1, 0:NG], lhsT=ones_col, rhs=sq[:],
                                start=(o == 0), stop=(o == DO - 1))
                            if o == 3:
                                yield None
                        mean_n = lns.tile([1, NG], F32, tag="mn", name=f"{tagp}_mn")
                        nc.vector.tensor_scalar(
                            out=mean_n[:], in0=ps_s[0:1, 0:NG], scalar1=-1.0 / D,
                            scalar2=None, op0=ALU.mult)
                        msq_e = lns.tile([1, NG], F32, tag="me", name=f"{tagp}_me")
                        nc.vector.tensor_scalar(
                            out=msq_e[:], in0=ps_q[0:1, 0:NG], scalar1=1.0 / D,
                            scalar2=1e-5, op0=ALU.mult, op1=ALU.add)
                        m2 = lns.tile([1, NG], F32, tag="m2", name=f"{tagp}_m2")
                        nc.vector.tensor_mul(m2[:], mean_n[:], mean_n[:])
                        var = lns.tile([1, NG], F32, tag="va", name=f"{tagp}_va")
                        nc.vector.scalar_tensor_tensor(
                            out=var[:], in0=m2[:], scalar=-1.0, in1=msq_e[:],
                            op0=ALU.mult, op1=ALU.add)
                        lv = lns.tile([1, NG], F32, tag="lv", name=f"{tagp}_lv")
                        nc.scalar.activation(out=lv[:], in_=var[:], func=AF.Ln)
                        rstd = lns.tile([1, NG], F32, tag="rs", name=f"{tagp}_rs")
                        nc.scalar.activation(out=rstd[:], in_=lv[:],
                                             func=AF.Exp, scale=-0.5)
                        negms = lns.tile([1, NG], F32, tag="ng", name=f"{tagp}_ng")
                        nc.vector.tensor_mul(negms[:], mean_n[:], rstd[:])
                        rstd_b = bcp.tile([P, NG], F32, tag="lnb",
                                          name=f"{tagp}_rb")
                        nc.gpsimd.partition_broadcast(rstd_b[:], rstd[:])
                        negms_b = bcp.tile([P, NG], F32, tag="lnb",
                                           name=f"{tagp}_nb")
                        nc.gpsimd.partition_broadcast(negms_b[:], negms[:])
                        yield (rstd_b, negms_b)

                    def ffn_gen(g, ctx_t):
                        sl = bass.ts(g, NG)
                        xq_sb = ffs2.tile([P, DO, NG], BF16, tag="xq",
                                          name=f"xq_{g}")
                        nc.sync.dma_start(out=xq_sb[:], in_=xtb3[:, :, sl])
                        w1_t = [None, None]
                        w2_t = [None, None]

                        def load_w1(c):
                            w1_t[c % 2] = w12.tile([P, DO, 512], BF16, tag="w1",
                                                   name=f"w1_{g}_{c}")
                            nc.gpsimd.dma_start(out=w1_t[c % 2][:], in_=w1s[c])

                        def load_w2(m):
                            w2_t[m % 2] = w12.tile([P, FO, P], BF16, tag="w2",
                                                   name=f"w2_{g}_{m}")
                            nc.gpsimd.dma_start(out=w2_t[m % 2][:], in_=w2s[m])

                        load_w1(0)
                        load_w1(1)
                        hA = ffh1.tile([P, DO, NG], BF16, tag="hA",
                                       name=f"hA_{g}")
                        hB = ffh1.tile([P, DO, NG], BF16, tag="hB",
                                       name=f"hB_{g}")
                        # w_o (fp8 DoubleRow) + residual (bo pre-folded into xtb)
                        for m in range(DO):
                            ps = mmp.tile([P, NW], F32, tag="mm",
                                          name=f"wo_{g}_{m % 2}")
                            for kp in range(4):
                                nc.tensor.matmul(
                                    ps[:, 0:NG], lhsT=wo_sb[:, kp, :, bass.ts(m, P)],
                                    rhs=ctx_t[:, 2 * kp:2 * kp + 2, :],
                                    start=(kp == 0), stop=(kp == 3),
                                    perf_mode=DR)
                            nc.vector.scalar_tensor_tensor(
                                out=hA[:, m, :], in0=ps[:, 0:NG],
                                scalar=WO_SCALE, in1=xq_sb[:, m, :],
                                op0=ALU.mult, op1=ALU.add)
                            if m % 2 == 1:
                                yield
                        # LN1 (no affine; g1/be1 folded into w1/b1p/fc2 epilogue)
                        gst = ln_stats(hA, f"l1_{g}")
                        next(gst)
                        yield
                        rstd_b, negms_b = next(gst)
                        h1b = ffhb.tile([P, DO, NG], BF16, tag="h1b",
                                        name=f"h1b_{g}")
                        for o in range(DO):
                            nc.vector.tensor_mul(hA[:, o, :], hA[:, o, :],
                                                 rstd_b[:])
                            nc.vector.tensor_add(h1b[:, o, :], hA[:, o, :],
                                                 negms_b[:])
                            if o % 4 == 3:
                                yield
                        # fc1: aT = relu(w1g^T @ h1n + b1p)   (bf16)
                        aT = ffa.tile([P, FO, NG], BF16, tag="aT",
                                      name=f"aT_{g}")
                        for c in range(8):
                            w1c = w1_t[c % 2]
                            for u in range(4):
                                mf = 4 * c + u
                                ps = mmp.tile([P, NW], F32, tag="mm",
                                              name=f"f1_{g}_{mf % 2}")
                                for kc in range(DO):
                                    nc.tensor.matmul(
                                        ps[:, 0:NG],
                                        lhsT=w1c[:, kc, bass.ts(u, P)],
                                        rhs=h1b[:, kc, :],
                                        start=(kc == 0), stop=(kc == DO - 1))
                                nc.scalar.activation(
                                    out=aT[:, mf, :], in_=ps[:, 0:NG],
                                    func=AF.Relu, bias=b1p_sb[:, mf:mf + 1])
                            if c + 2 < 8:
                                load_w1(c + 2)
                            elif c == 6:
                                load_w2(0)
                            else:
                                load_w2(1)
                            yield
                        # fc2 + residual (h2 = h1n*g1 + be1 + ff + b2)
                        for m in range(DO):
                            w2c = w2_t[m % 2]
                            ps = mmp.tile([P, NW], F32, tag="mm",
                                          name=f"f2_{g}_{m % 2}")
                            for kc in range(FO):
                                nc.tensor.matmul(
                                    ps[:, 0:NG], lhsT=w2c[:, kc, :],
                                    rhs=aT[:, kc, :],
                                    start=(kc == 0), stop=(kc == FO - 1))
                            ep = work.tile([P, NG], F32, tag="ep",
                                           name=f"ep_{g}_{m % 2}")
                            nc.scalar.activation(out=ep[:], in_=ps[:, 0:NG],
                                                 func=AF.Identity,
                                                 bias=b2e_sb[:, m:m + 1])
                            nc.vector.scalar_tensor_tensor(
                                out=hB[:, m, :], in0=h1b[:, m, :],
                                scalar=g1_sb[:, m:m + 1], in1=ep[:],
                                op0=ALU.mult, op1=ALU.add)
                            if m + 2 < DO:
                                load_w2(m + 2)
                            yield
                        # LN2 -> yt
                        gst = ln_stats(hB, f"l2_{g}")
                        next(gst)
                        yield
                        rstd_b, negms_b = next(gst)
                        # write LN2 into scratch tiles (hB frees at the muls)
                        for o in range(DO):
                            tmp = work.tile([P, NG], F32R, tag="ep",
                                            name=f"yt_{g}_{o % 2}")
                            nc.vector.tensor_mul(tmp[:], hB[:, o, :],
                                                 rstd_b[:])
                            nc.vector.tensor_add(tmp[:], tmp[:], negms_b[:])
                            nc.scalar.activation(
                                out=tmp[:], in_=tmp[:], func=AF.Identity,
                                scale=g2_sb[:, o:o + 1],
                                bias=be2_sb[:, o:o + 1])
                            nc.sync.dma_start(out=yt3[:, o, sl],
                                              in_=tmp[:])
                            if o % 4 == 3:
                                yield

                    # ---- main interleaved emission ----
                    units = [(qn, j, half) for qn in range(NQA)
                             for j in range(H // 2) for half in range(2)]
                    gens = []

                    def drain(n):
                        k = 0
                        while k < n and gens:
                            try:
                                next(gens[0])
                                k += 1
                            except StopIteration:
                                gens.pop(0)

                    ctx_tiles = {}
                    pcs = None
                    prev = emit_scores(*units[0])
                    for i, (qn, j, half) in enumerate(units):
                        if (j, half) == (0, 0):
                            ctx_tiles[qn] = ctxp.tile(
                                [P, DO, NG], F8E4, tag="ctx",
                                name=f"ctx_{qn}")
                        cur = prev
                        if i + 1 < len(units):
                            prev = emit_scores(*units[i + 1])
                        if half == 0:
                            pcs = [pcp.tile([P, NF], F32, tag="pc",
                                            name=f"pc_{qn}_{j}_{i2}")
                                   for i2 in range(2)]
                        drain(2 if i >= 16 else 1)
                        emit_av(qn, j, half, cur, pcs, ctx_tiles[qn])
                        if (j, half) == (H // 2 - 1, 1):
                            gens.append(ffn_gen(qn, ctx_tiles[qn]))
                    drain(10 ** 6)

    nc.compile()
    return nc


_CACHE = {}


def _compiled():
    if "nc" not in _CACHE:
        _CACHE["nc"] = build()
    return _CACHE["nc"]


def make_in_maps(x, w_q, b_q, w_k, b_k, w_v, b_v, w_o, b_o,
                 w1, b1, w2, b2, g1, be1, g2, be2):
    bf = ml_dtypes.bfloat16
    f8 = ml_dtypes.float8_e4m3
    x = np.asarray(x, np.float32)
    f32 = lambda a: np.ascontiguousarray(np.asarray(a, np.float32))

    def w_dr(w, scale):
        # [D, M] -> [P, 4, 2, M] fp8 with contraction chunk pairs interleaved
        a = np.clip(f32(w) * scale, -240.0, 240.0)
        return np.ascontiguousarray(
            a.reshape(4, 2, P, -1).transpose(2, 0, 1, 3)).astype(f8)

    w_o32 = f32(w_o)
    w1_32 = f32(w1)
    g1_32 = f32(g1)
    be1_32 = f32(be1)
    w1g = w1_32 * g1_32[:, None]
    w1r = np.ascontiguousarray(
        w1g.reshape(DO, P, FF).transpose(1, 0, 2))          # [P, DO, FF]
    w1s_h = np.ascontiguousarray(
        w1r.reshape(P, DO, 8, 512).transpose(2, 0, 1, 3)).astype(bf)
    w2_32 = f32(w2)
    w2r = np.ascontiguousarray(
        w2_32.reshape(FO, P, D).transpose(1, 0, 2))          # [P, FO, D]
    w2s_h = np.ascontiguousarray(
        w2r.reshape(P, FO, DO, P).transpose(2, 0, 1, 3)).astype(bf)

    bo = f32(b_o) + f32(b_v) @ w_o32
    shared = {
        "wq8": w_dr(w_q, SW), "wk8": w_dr(w_k, SW),
        "wv8": w_dr(w_v, SW), "wo8": w_dr(w_o, SW),
        "w1s": w1s_h, "w2s": w2s_h,
        "bq2": f32(b_q) * SQ, "bk2": f32(b_k) * SQ,
        "b1p": f32(b1) + be1_32 @ w1_32,
        "b2e": f32(b2) + be1_32,
        "g1": g1_32, "g2": f32(g2), "be2": f32(be2),
        "onesr": np.ones((P,), ml_dtypes.bfloat16),
    }
    in_maps = []
    for c in range(8):
        b, r = c // 2, c % 2
        xb = x[b]
        xc = np.concatenate([xb[r * TQ:(r + 1) * TQ],
                             xb[(1 - r) * TQ:(2 - r) * TQ]], axis=0)
        m = dict(shared)
        xcT = np.ascontiguousarray(xc.T)                     # [D, S]
        m["xt8"] = np.ascontiguousarray(
            np.clip(xcT * SX, -240.0, 240.0)
            .reshape(4, 2, P, S).transpose(2, 0, 1, 3)).astype(f8)
        m["xtb"] = np.ascontiguousarray(
            (xb[r * TQ:(r + 1) * TQ] + bo[None, :]).T).astype(bf)
        in_maps.append(m)
    return in_maps


def assemble_out(results):
    out = np.empty((4, 2048, 1024), np.float32)
    for c in range(8):
        b, r = c // 2, c % 2
        out[b, r * TQ:(r + 1) * TQ] = results[c]["yt"].T
    return out


def kernel(x, src_mask, w_q, b_q, w_k, b_k, w_v, b_v, w_o, b_o,
           w1, b1, w2, b2, g1, be1, g2, be2):
    in_maps = make_in_maps(x, w_q, b_q, w_k, b_k, w_v, b_v, w_o, b_o,
                           w1, b1, w2, b2, g1, be1, g2, be2)
    nc = _compiled()
    res = run_bass_kernel_spmd(nc, in_maps, core_ids=list(range(8)))
    return assemble_out(res.results)


# revision 35
# speedup vs baseline: 1.0020x; 1.0020x over previous
"""Trainium2 Bass kernel: transformer encoder layer (B=4, S=2048, D=1024, H=16, FF=4096).

Sharding (8 NeuronCores, no collectives): core c handles batch b=c//2 and
query-token half r=c%2 (1024 query rows). K/V are recomputed per core over the
batch's full 2048-token sequence. All activations feature-on-partition.

fp8 everywhere the error budget allows (tolerance 2e-2; this lands ~6e-3):
- QKV/w_o projections run fp8-e4m3 DoubleRow (contraction-chunk pairs
  interleaved in the free dim -> 2x PE rate). Host pre-scales by powers of 2
  (x*16, w*64); epilogues rescale via ACT scale / DVE tensor_scalar.
- Softmax probs are fp8-e5m2: ScalarE writes Exp directly to fp8; VectorE uses
  an int8 Schraudolph (round(s*A+B) bitcast e5m2). A/V matmul is fp8 DoubleRow
  over kv-chunk pairs; V is e4m3*8. The softmax denominator rides as a 65th V
  column = 0.25, so 1/den is directly the ctx normalizer at ctx scale 32.
- Attention runs 512-wide query chunks; the per-kc scores PSUM tile packs both
  heads of a pair as free-dim planes so one [P,1024] exp op covers both, with
  exp work split ~50/50 between ScalarE and VectorE.
- FFN stays bf16 (fp8 there would blow the error budget) in two 512-wide
  groups; each group's FFN is a generator whose PE slices are drained one per
  attention unit, hiding exp latency under FFN matmuls. Residual stream is
  bf16 (hA: wo+residual -> LN1; h1b: LN1 out; hB: fc2+residual -> LN2) with
  separate tiles per role so group boundaries carry no WAR stalls.
- b_o + b_v@w_o is folded into the DMA'd residual (xtb = x + bo); b_q/b_k/b1/
  b2/be1 and g1 are folded into biases/weights on the host.
"""

import numpy as np
import ml_dtypes

import concourse.bass as bass
import concourse.tile as tile
from concourse import bacc
from concourse import mybir
from concourse.bass_utils import run_bass_kernel_spmd

P = 128
D = 1024          # d_model
S = 2048          # kv sequence length per core (one full batch)
TQ = 1024         # query tokens per core
H = 16            # heads
DK = 64           # head dim
FF = 4096         # ffn dim
DO = D // P       # 8  d_model chunks
KC = S // P       # 16 kv-token chunks
FO = FF // P      # 32 ffn chunks
NF = 512          # attention query-chunk width
NQA = TQ // NF    # 2 attention chunks
NG = 512          # ffn group width (2 attention chunks)
NW = 512          # projection free-dim tile
KH = KC // 2      # kv chunks per half
KP = KH // 2      # kv chunk-pairs per half

# fp8 power-of-2 scales
SX, SW, SQ, SV, SCTX = 16.0, 64.0, 2.0, 8.0, 32.0
KQ_SCALE = SQ / (SX * SW)        # 2^-9 on the QKV psum
V_SCALE = SV / (SX * SW)         # 2^-7
WO_SCALE = 1.0 / (SCTX * SW)     # 2^-11
EXP_SCALE = 1.0 / (8.0 * SQ * SQ)  # scores psum = 4*qk; exp arg = qk/8

# Schraudolph exp-to-e5m2: bits8 = round(s*A + B); exp(s*EXP_SCALE) ~ bitcast
SCH5_A = 4.0 * 1.4426950408889634 * EXP_SCALE   # 2^2 mantissa steps / octave
SCH5_B = 15.0 * 4.0 - 0.2316                    # exp-bias*4 - schraudolph corr

BF16 = mybir.dt.bfloat16
F32 = mybir.dt.float32
F32R = mybir.dt.float32r
F8E4 = mybir.dt.float8e4
F8E5 = mybir.dt.float8e5
I8 = mybir.dt.int8
AF = mybir.ActivationFunctionType
ALU = mybir.AluOpType
DR = mybir.MatmulPerfMode.DoubleRow


def build():
    nc = bacc.Bacc("TRN2", target_bir_lowering=False, debug=False, num_devices=8)

    xt8 = nc.dram_tensor("xt8", [P, 4, 2, S], F8E4, kind="ExternalInput").ap()
    wq8 = nc.dram_tensor("wq8", [P, 4, 2, D], F8E4, kind="ExternalInput").ap()
    wk8 = nc.dram_tensor("wk8", [P, 4, 2, D], F8E4, kind="ExternalInput").ap()
    wv8 = nc.dram_tensor("wv8", [P, 4, 2, D], F8E4, kind="ExternalInput").ap()
    wo8 = nc.dram_tensor("wo8", [P, 4, 2, D], F8E4, kind="ExternalInput").ap()
    xtb = nc.dram_tensor("xtb", [D, TQ], BF16, kind="ExternalInput").ap()
    w1s = nc.dram_tensor("w1s", [8, P, DO, 512], BF16, kind="ExternalInput").ap()
    w2s = nc.dram_tensor("w2s", [8, P, FO, P], BF16, kind="ExternalInput").ap()
    bq2 = nc.dram_tensor("bq2", [D], F32, kind="ExternalInput").ap()   # 2*b_q
    bk2 = nc.dram_tensor("bk2", [D], F32, kind="ExternalInput").ap()   # 2*b_k
    b1p = nc.dram_tensor("b1p", [FF], F32, kind="ExternalInput").ap()  # b1 + be1@w1
    b2e = nc.dram_tensor("b2e", [D], F32, kind="ExternalInput").ap()   # b2 + be1
    g1 = nc.dram_tensor("g1", [D], F32, kind="ExternalInput").ap()
    g2 = nc.dram_tensor("g2", [D], F32, kind="ExternalInput").ap()
    be2 = nc.dram_tensor("be2", [D], F32, kind="ExternalInput").ap()
    onesr = nc.dram_tensor("onesr", [P], BF16, kind="ExternalInput").ap()
    yt = nc.dram_tensor("yt", [D, TQ], F32R, kind="ExternalOutput").ap()

    xtb3 = xtb.rearrange("(o p) t -> p o t", p=P)
    yt3 = yt.rearrange("(o p) t -> p o t", p=P)

    with tile.TileContext(nc) as tc:
        with (
            tc.tile_pool(name="persist", bufs=1) as persist,
            tc.tile_pool(name="lns", bufs=1) as lns,
            tc.tile_pool(name="recp", bufs=2) as recp,
            tc.tile_pool(name="work", bufs=2) as work,
            tc.tile_pool(name="bcp", bufs=2) as bcp,
            tc.tile_pool(name="mm", bufs=2, space="PSUM") as mmp,
            tc.tile_pool(name="pssc", bufs=2, space="PSUM") as pssc,
            tc.tile_pool(name="pcp", bufs=2, space="PSUM") as pcp,
        ):
            def load_vec(ap, n_chunks, name):
                t = persist.tile([P, n_chunks], F32, tag=name)
                nc.sync.dma_start(out=t[:], in_=ap.rearrange("(o p) -> p o", p=P))
                return t

            eng_flip = [0]

            with tc.tile_pool(name="kqv", bufs=1) as kqvp:
                kT = kqvp.tile([P, DO, S], F8E4, tag="kT")
                qT = kqvp.tile([P, DO, TQ], F8E4, tag="qT")
                vaug = kqvp.tile([P, KC // 2, 2, H * 65], F8E4, tag="vaug")
                vaug_h = vaug.rearrange("p g l (h w) -> p g l h w", w=65)

                # ---------------- phase 1: QKV projections (fp8 DoubleRow) ----
                with (
                    tc.tile_pool(name="xtp", bufs=1) as xtp,
                    tc.tile_pool(name="wst", bufs=2) as wst,
                ):
                    wk_sb = wst.tile([P, 4, 2, D], F8E4, tag="w", name="w_k")
                    nc.sync.dma_start(out=wk_sb[:], in_=wk8)
                    xt_sb = xtp.tile([P, 4, 2, S], F8E4, tag="xt")
                    for n in range(S // NW):
                        nc.sync.dma_start(out=xt_sb[:, :, :, bass.ts(n, NW)],
                                          in_=xt8[:, :, :, bass.ts(n, NW)])
                    bq_sb = load_vec(bq2, DO, "bq")
                    bk_sb = load_vec(bk2, DO, "bk")
                    b2e_sb = load_vec(b2e, DO, "b2e")
                    g1_sb = load_vec(g1, DO, "g1")
                    g2_sb = load_vec(g2, DO, "g2")
                    be2_sb = load_vec(be2, DO, "be2")
                    b1p_sb = load_vec(b1p, FO, "b1p")
                    ones_col = persist.tile([P, 1], BF16, tag="ones_col")
                    nc.sync.dma_start(out=ones_col[:], in_=onesr[:, None])
                    for t in range(KC):
                        nc.vector.memset(vaug_h[:, t // 2, t % 2, :, 64:65], 0.25)

                    def proj_epilogue(out_ap, ps, bias_col, scale):
                        # alternate ACT/DVE so both engines share QKV epilogues
                        eng_flip[0] ^= 1
                        if eng_flip[0]:
                            if bias_col is None:
                                nc.scalar.activation(out=out_ap, in_=ps,
                                                     func=AF.Identity, scale=scale)
                            else:
                                nc.scalar.activation(out=out_ap, in_=ps,
                                                     func=AF.Identity,
                                                     bias=bias_col, scale=scale)
                        else:
                            if bias_col is None:
                                nc.vector.tensor_scalar(
                                    out=out_ap, in0=ps, scalar1=scale,
                                    scalar2=None, op0=ALU.mult)
                            else:
                                nc.vector.tensor_scalar(
                                    out=out_ap, in0=ps, scalar1=scale,
                                    scalar2=bias_col, op0=ALU.mult, op1=ALU.add)

                    def kq_slice(w_sb, out_sb, bias_sb, n, m, nm):
                        ps = mmp.tile([P, NW], F32, tag="mm", name=f"mm_{nm}")
                        for kp in range(4):
                            nc.tensor.matmul(
                                ps[:], lhsT=w_sb[:, kp, :, bass.ts(m, P)],
                                rhs=xt_sb[:, kp, :, bass.ts(n, NW)],
                                start=(kp == 0), stop=(kp == 3),
                                perf_mode=DR)
                        proj_epilogue(out_sb[:, m, bass.ts(n, NW)], ps[:],
                                      bias_sb[:, m:m + 1], KQ_SCALE)

                    for n in range(S // NW):
                        for m in range(DO):
                            kq_slice(wk_sb, kT, bk_sb, n, m, f"k{n}_{m}")
                    wq_sb = wst.tile([P, 4, 2, D], F8E4, tag="w", name="w_q")
                    nc.sync.dma_start(out=wq_sb[:], in_=wq8)
                    for n in range(TQ // NW):
                        for m in range(DO):
                            kq_slice(wq_sb, qT, bq_sb, n, m, f"q{n}_{m}")

                    # V in natural [token, d] layout, heads padded to 65 cols
                    wv_sb = wst.tile([P, 4, 2, D], F8E4, tag="w", name="w_v")
                    nc.sync.dma_start(out=wv_sb[:], in_=wv8)
                    for t in range(KC):
                        for n in range(2):
                            ps = mmp.tile([P, NW], F32, tag="mm",
                                          name=f"mmv_{t}_{n}")
                            for kp in range(4):
                                nc.tensor.matmul(
                                    ps[:], lhsT=xt_sb[:, kp, :, bass.ts(t, P)],
                                    rhs=wv_sb[:, kp, :, bass.ts(n, NW)],
                                    start=(kp == 0), stop=(kp == 3),
                                    perf_mode=DR)
                            psh = ps.rearrange("p (h w) -> p h w", w=DK)
                            dst = vaug_h[:, t // 2, t % 2, 8 * n:8 * n + 8, 0:64]
                            proj_epilogue(dst, psh, None, V_SCALE)

                # ------------- phase 2: attention + interleaved FFN -----------
                with (
                    tc.tile_pool(name="es", bufs=2) as esp,
                    tc.tile_pool(name="ctxp", bufs=2) as ctxp,
                    tc.tile_pool(name="wop", bufs=1) as wop,
                    tc.tile_pool(name="w12", bufs=2) as w12,
                    tc.tile_pool(name="ffs2", bufs=1) as ffs2,
                    tc.tile_pool(name="ffh1", bufs=1) as ffh1,
                    tc.tile_pool(name="ffhb", bufs=1) as ffhb,
                    tc.tile_pool(name="ffa", bufs=1) as ffa,
                ):
                    wo_sb = wop.tile([P, 4, 2, D], F8E4, tag="wo")
                    nc.sync.dma_start(out=wo_sb[:], in_=wo8)

                    exp_acc = [0.0]
                    ACT_FRAC = 0.58

                    def emit_exp(dst_f8, ps):
                        """softmax exp of one [P,2,NF] scores tile -> e5m2."""
                        exp_acc[0] += ACT_FRAC
                        if exp_acc[0] >= 1.0:
                            exp_acc[0] -= 1.0
                            nc.scalar.activation(out=dst_f8, in_=ps[:],
                                                 func=AF.Exp, scale=EXP_SCALE)
                        else:
                            nc.vector.tensor_scalar(
                                out=dst_f8.bitcast(I8), in0=ps[:],
                                scalar1=SCH5_A, scalar2=SCH5_B,
                                op0=ALU.mult, op1=ALU.add)

                    def emit_scores(qn, j, half):
                        """scores+exp for 8 kv chunks x both heads of pair j.
                        es layout [P, idx, KH, NF]; psum tiles pack idx0/idx1
                        as free-dim planes so one [P,1024] exp covers both."""
                        qsl = bass.ts(qn, NF)
                        es_u = esp.tile([P, 2, KH, NF], F8E5, tag="es",
                                        name=f"es_{qn}_{j}_{half}")
                        for kl in range(KH):
                            kc = half * KH + kl
                            pss = pssc.tile([P, 2, NF], F32, tag="sc",
                                            name=f"sc_{kl % 2}")
                            for idx in range(2):
                                off = idx * DK
                                nc.tensor.matmul(
                                    pss[:, idx, :],
                                    lhsT=kT[off:off + DK, j, bass.ts(kc, P)],
                                    rhs=qT[off:off + DK, j, qsl],
                                    start=True, stop=True)
                            emit_exp(es_u[:, :, kl, :], pss)
                        return es_u

                    def emit_av(qn, j, half, es_u, pcs, ctx_t):
                        for idx in range(2):
                            h = 2 * j + idx
                            pc = pcs[idx]
                            for kp in range(KP):
                                pg = half * KP + kp
                                nc.tensor.matmul(
                                    pc[0:DK + 1, :],
                                    lhsT=vaug[:, pg, :, h * 65:(h + 1) * 65],
                                    rhs=es_u[:, idx, 2 * kp:2 * kp + 2, :],
                                    start=(pg == 0), stop=(pg == 2 * KP - 1),
                                    perf_mode=DR)
                            if half == 0:
                                continue
                            den = lns.tile([1, NF], F32, tag="den",
                                           name=f"den_{qn}_{h}")
                            nc.vector.tensor_copy(den[:], pc[DK:DK + 1, :])
                            rec = recp.tile([1, NF], F32, tag="rec",
                                            name=f"rec_{qn}_{h}")
                            nc.vector.reciprocal_approx_fast(
                                out=rec[:], in_=den[:])
                            recb = bcp.tile([DK, NF], F32, tag="recb",
                                            name=f"recb_{qn}_{h}")
                            nc.gpsimd.partition_broadcast(recb[:], rec[:],
                                                          channels=DK)
                            nc.vector.tensor_mul(
                                ctx_t[idx * DK:(idx + 1) * DK, j, :],
                                pc[0:DK, :], recb[:])

                    # ---- FFN for one 512-wide group, as PE-slice generator ----
                    def ln_stats(h3, tagp):
                        """LN over the partition (feature) dim of h3 [P,DO,NG].
                        sum in psum row 0, sumsq in row 32 (col-packed).
                        Yields once mid-way; returns (rstd_b, negms_b)."""
                        ps_s = mmp.tile([P, NW], F32, tag="mm", name=f"{tagp}_s")
                        ps_q = mmp.tile([P, NW], F32, tag="mm", name=f"{tagp}_q")
                        for o in range(DO):
                            nc.tensor.matmul(
                                ps_s[0:1, 0:NG], lhsT=ones_col,
                                rhs=h3[:, o, :],
                                start=(o == 0), stop=(o == DO - 1))
                            sq = work.tile([P, NG], BF16, tag="sq",
                                           name=f"{tagp}_sq{o % 2}")
                            nc.scalar.activation(out=sq[:], in_=h3[:, o, :],
                                                 func=AF.Square)
                            nc.tensor.matmul(
                                ps_q[0:1, 0:NG], lhsT=ones_col, rhs=sq[:],
                                start=(o == 0), stop=(o == DO - 1))
                            if o == 3:
                                yield None
                        mean_n = lns.tile([1, NG], F32, tag="mn", name=f"{tagp}_mn")
                        nc.vector.tensor_scalar(
                            out=mean_n[:], in0=ps_s[0:1, 0:NG], scalar1=-1.0 / D,
                            scalar2=None, op0=ALU.mult)
                        msq_e = lns.tile([1, NG], F32, tag="me", name=f"{tagp}_me")
                        nc.vector.tensor_scalar(
                            out=msq_e[:], in0=ps_q[0:1, 0:NG], scalar1=1.0 / D,
                            scalar2=1e-5, op0=ALU.mult, op1=ALU.add)
                        m2 = lns.tile([1, NG], F32, tag="m2", name=f"{tagp}_m2")
                        nc.vector.tensor_mul(m2[:], mean_n[:], mean_n[:])
                        var = lns.tile([1, NG], F32, tag="va", name=f"{tagp}_va")
                        nc.vector.scalar_tensor_tensor(
                            out=var[:], in0=m2[:], scalar=-1.0, in1=msq_e[:],
                            op0=ALU.mult, op1=ALU.add)
                        lv = lns.tile([1, NG], F32, tag="lv", name=f"{tagp}_lv")
                        nc.scalar.activation(out=lv[:], in_=var[:], func=AF.Ln)
                        rstd = lns.tile([1, NG], F32, tag="rs", name=f"{tagp}_rs")
                        nc.scalar.activation(out=rstd[:], in_=lv[:],
                                             func=AF.Exp, scale=-0.5)
                        negms = lns.tile([1, NG], F32, tag="ng", name=f"{tagp}_ng")
                        nc.vector.tensor_mul(negms[:], mean_n[:], rstd[:])
                        rstd_b = bcp.tile([P, NG], F32, tag="lnb",
                                          name=f"{tagp}_rb")
                        nc.gpsimd.partition_broadcast(rstd_b[:], rstd[:])
                        negms_b = bcp.tile([P, NG], F32, tag="lnb",
                                           name=f"{tagp}_nb")
                        nc.gpsimd.partition_broadcast(negms_b[:], negms[:])
                        yield (rstd_b, negms_b)

                    def ffn_gen(g, ctx_t):
                        sl = bass.ts(g, NG)
                        xq_sb = ffs2.tile([P, DO, NG], BF16, tag="xq",
                                          name=f"xq_{g}")
                        nc.sync.dma_start(out=xq_sb[:], in_=xtb3[:, :, sl])
                        w1_t = [None, None]
                        w2_t = [None, None]

                        def load_w1(c):
                            w1_t[c % 2] = w12.tile([P, DO, 512], BF16, tag="w1",
                                                   name=f"w1_{g}_{c}")
                            nc.gpsimd.dma_start(out=w1_t[c % 2][:], in_=w1s[c])

                        def load_w2(m):
                            w2_t[m % 2] = w12.tile([P, FO, P], BF16, tag="w2",
                                                   name=f"w2_{g}_{m}")
                            nc.gpsimd.dma_start(out=w2_t[m % 2][:], in_=w2s[m])

                        load_w1(0)
                        load_w1(1)
                        hA = ffh1.tile([P, DO, NG], BF16, tag="hA",
                                       name=f"hA_{g}")
                        hB = ffh1.tile([P, DO, NG], BF16, tag="hB",
                                       name=f"hB_{g}")
                        # w_o (fp8 DoubleRow) + residual (bo pre-folded into xtb)
                        for m in range(DO):
                            ps = mmp.tile([P, NW], F32, tag="mm",
                                          name=f"wo_{g}_{m % 2}")
                            for kp in range(4):
                                nc.tensor.matmul(
                                    ps[:, 0:NG], lhsT=wo_sb[:, kp, :, bass.ts(m, P)],
                                    rhs=ctx_t[:, 2 * kp:2 * kp + 2, :],
                                    start=(kp == 0), stop=(kp == 3),
                                    perf_mode=DR)
                            nc.vector.scalar_tensor_tensor(
                                out=hA[:, m, :], in0=ps[:, 0:NG],
                                scalar=WO_SCALE, in1=xq_sb[:, m, :],
                                op0=ALU.mult, op1=ALU.add)
                            if m % 2 == 1:
                                yield
                        # LN1 (no affine; g1/be1 folded into w1/b1p/fc2 epilogue)
                        gst = ln_stats(hA, f"l1_{g}")
                        next(gst)
                        yield
                        rstd_b, negms_b = next(gst)
                        h1b = ffhb.tile([P, DO, NG], BF16, tag="h1b",
                                        name=f"h1b_{g}")
                        for o in range(DO):
                            nc.vector.tensor_mul(hA[:, o, :], hA[:, o, :],
                                                 rstd_b[:])
                            nc.vector.tensor_add(h1b[:, o, :], hA[:, o, :],
                                                 negms_b[:])
                            if o % 4 == 3:
                                yield
                        # fc1: aT = relu(w1g^T @ h1n + b1p)   (bf16)
                        aT = ffa.tile([P, FO, NG], BF16, tag="aT",
                                      name=f"aT_{g}")
                        for c in range(8):
                            w1c = w1_t[c % 2]
                            for u in range(4):
                                mf = 4 * c + u
                                ps = mmp.tile([P, NW], F32, tag="mm",
                                              name=f"f1_{g}_{mf % 2}")
                                for kc in range(DO):
                                    nc.tensor.matmul(
                                        ps[:, 0:NG],
                                        lhsT=w1c[:, kc, bass.ts(u, P)],
                                        rhs=h1b[:, kc, :],
                                        start=(kc == 0), stop=(kc == DO - 1))
                                nc.scalar.activation(
                                    out=aT[:, mf, :], in_=ps[:, 0:NG],
                                    func=AF.Relu, bias=b1p_sb[:, mf:mf + 1])
                            if c + 2 < 8:
                                load_w1(c + 2)
                            elif c == 6:
                                load_w2(0)
                            else:
                                load_w2(1)
                            yield
                        # fc2 + residual (h2 = h1n*g1 + be1 + ff + b2)
                        for m in range(DO):
                            w2c = w2_t[m % 2]
                            ps = mmp.tile([P, NW], F32, tag="mm",
                                          name=f"f2_{g}_{m % 2}")
                            for kc in range(FO):
                                nc.tensor.matmul(
                                    ps[:, 0:NG], lhsT=w2c[:, kc, :],
                                    rhs=aT[:, kc, :],
                                    start=(kc == 0), stop=(kc == FO - 1))
                            ep = work.tile([P, NG], F32, tag="ep",
                                           name=f"ep_{g}_{m % 2}")
                            nc.scalar.activation(out=ep[:], in_=ps[:, 0:NG],
                                                 func=AF.Identity,
                                                 bias=b2e_sb[:, m:m + 1])
                            nc.vector.scalar_tensor_tensor(
                                out=hB[:, m, :], in0=h1b[:, m, :],
                                scalar=g1_sb[:, m:m + 1], in1=ep[:],
                                op0=ALU.mult, op1=ALU.add)
                            if m + 2 < DO:
                                load_w2(m + 2)
                            yield
                        # LN2 -> yt
                        gst = ln_stats(hB, f"l2_{g}")
                        next(gst)
                        yield
                        rstd_b, negms_b = next(gst)
                        # write LN2 into scratch tiles (hB frees at the muls)
                        for o in range(DO):
                            tmp = work.tile([P, NG], F32R, tag="ep",
                                            name=f"yt_{g}_{o % 2}")
                            nc.vector.tensor_mul(tmp[:], hB[:, o, :],
                                                 rstd_b[:])
                            nc.vector.tensor_add(tmp[:], tmp[:], negms_b[:])
                            nc.scalar.activation(
                                out=tmp[:], in_=tmp[:], func=AF.Identity,
                                scale=g2_sb[:, o:o + 1],
                                bias=be2_sb[:, o:o + 1])
                            nc.sync.dma_start(out=yt3[:, o, sl],
                                              in_=tmp[:])
                            if o % 4 == 3:
                                yield

                    # ---- main interleaved emission ----
                    units = [(qn, j, half) for qn in range(NQA)
                             for j in range(H // 2) for half in range(2)]
                    gens = []

                    def drain(n):
                        k = 0
                        while k < n and gens:
                            try:
                                next(gens[0])
                                k += 1
                            except StopIteration:
                                gens.pop(0)

                    ctx_tiles = {}
                    pcs = None
                    prev = emit_scores(*units[0])
                    for i, (qn, j, half) in enumerate(units):
                        if (j, half) == (0, 0):
                            ctx_tiles[qn] = ctxp.tile(
                                [P, DO, NG], F8E4, tag="ctx",
                                name=f"ctx_{qn}")
                        cur = prev
                        if i + 1 < len(units):
                            prev = emit_scores(*units[i + 1])
                        if half == 0:
                            pcs = [pcp.tile([P, NF], F32, tag="pc",
                                            name=f"pc_{qn}_{j}_{i2}")
                                   for i2 in range(2)]
                        drain(2 if i >= 24 else 1)
                        emit_av(qn, j, half, cur, pcs, ctx_tiles[qn])
                        if (j, half) == (H // 2 - 1, 1):
                            gens.append(ffn_gen(qn, ctx_tiles[qn]))
                    drain(10 ** 6)

    nc.compile()
    return nc


_CACHE = {}


def _compiled():
    if "nc" not in _CACHE:
        _CACHE["nc"] = build()
    return _CACHE["nc"]


def make_in_maps(x, w_q, b_q, w_k, b_k, w_v, b_v, w_o, b_o,
                 w1, b1, w2, b2, g1, be1, g2, be2):
    bf = ml_dtypes.bfloat16
    f8 = ml_dtypes.float8_e4m3
    x = np.asarray(x, np.float32)
    f32 = lambda a: np.ascontiguousarray(np.asarray(a, np.float32))

    def w_dr(w, scale):
        # [D, M] -> [P, 4, 2, M] fp8 with contraction chunk pairs interleaved
        a = np.clip(f32(w) * scale, -240.0, 240.0)
        return np.ascontiguousarray(
            a.reshape(4, 2, P, -1).transpose(2, 0, 1, 3)).astype(f8)

    w_o32 = f32(w_o)
    w1_32 = f32(w1)
    g1_32 = f32(g1)
    be1_32 = f32(be1)
    w1g = w1_32 * g1_32[:, None]
    w1r = np.ascontiguousarray(
        w1g.reshape(DO, P, FF).transpose(1, 0, 2))          # [P, DO, FF]
    w1s_h = np.ascontiguousarray(
        w1r.reshape(P, DO, 8, 512).transpose(2, 0, 1, 3)).astype(bf)
    w2_32 = f32(w2)
    w2r = np.ascontiguousarray(
        w2_32.reshape(FO, P, D).transpose(1, 0, 2))          # [P, FO, D]
    w2s_h = np.ascontiguousarray(
        w2r.reshape(P, FO, DO, P).transpose(2, 0, 1, 3)).astype(bf)

    bo = f32(b_o) + f32(b_v) @ w_o32
    shared = {
        "wq8": w_dr(w_q, SW), "wk8": w_dr(w_k, SW),
        "wv8": w_dr(w_v, SW), "wo8": w_dr(w_o, SW),
        "w1s": w1s_h, "w2s": w2s_h,
        "bq2": f32(b_q) * SQ, "bk2": f32(b_k) * SQ,
        "b1p": f32(b1) + be1_32 @ w1_32,
        "b2e": f32(b2) + be1_32,
        "g1": g1_32, "g2": f32(g2), "be2": f32(be2),
        "onesr": np.ones((P,), ml_dtypes.bfloat16),
    }
    in_maps = []
    for c in range(8):
        b, r = c // 2, c % 2
        xb = x[b]
        xc = np.concatenate([xb[r * TQ:(r + 1) * TQ],
                             xb[(1 - r) * TQ:(2 - r) * TQ]], axis=0)
        m = dict(shared)
        xcT = np.ascontiguousarray(xc.T)                     # [D, S]
        m["xt8"] = np.ascontiguousarray(
            np.clip(xcT * SX, -240.0, 240.0)
            .reshape(4, 2, P, S).transpose(2, 0, 1, 3)).astype(f8)
        m["xtb"] = np.ascontiguousarray(
            (xb[r * TQ:(r + 1) * TQ] + bo[None, :]).T).astype(bf)
        in_maps.append(m)
    return in_maps


def assemble_out(results):
    out = np.empty((4, 2048, 1024), np.float32)
    for c in range(8):
        b, r = c // 2, c % 2
        out[b, r * TQ:(r + 1) * TQ] = results[c]["yt"].T
    return out


def kernel(x, src_mask, w_q, b_q, w_k, b_k, w_v, b_v, w_o, b_o,
           w1, b1, w2, b2, g1, be1, g2, be2):
    in_maps = make_in_maps(x, w_q, b_q, w_k, b_k, w_v, b_v, w_o, b_o,
                           w1, b1, w2, b2, g1, be1, g2, be2)
    nc = _compiled()
    res = run_bass_kernel_spmd(nc, in_maps, core_ids=list(range(8)))
    return assemble_out(res.results)


# revision 36
# speedup vs baseline: 1.0088x; 1.0068x over previous
"""Trainium2 Bass kernel: transformer encoder layer (B=4, S=2048, D=1024, H=16, FF=4096).

Sharding (8 NeuronCores, no collectives): core c handles batch b=c//2 and
query-token half r=c%2 (1024 query rows). K/V are recomputed per core over the
batch's full 2048-token sequence. All activations feature-on-partition.

fp8 everywhere the error budget allows (tolerance 2e-2; this lands ~6e-3):
- QKV/w_o projections run fp8-e4m3 DoubleRow (contraction-chunk pairs
  interleaved in the free dim -> 2x PE rate). Host pre-scales by powers of 2
  (x*16, w*64); epilogues rescale via ACT scale / DVE tensor_scalar.
- Softmax probs are fp8-e5m2: ScalarE writes Exp directly to fp8; VectorE uses
  an int8 Schraudolph (round(s*A+B) bitcast e5m2). A/V matmul is fp8 DoubleRow
  over kv-chunk pairs; V is e4m3*8. The softmax denominator rides as a 65th V
  column = 0.25, so 1/den is directly the ctx normalizer at ctx scale 32.
- Attention runs 512-wide query chunks; the per-kc scores PSUM tile packs both
  heads of a pair as free-dim planes so one [P,1024] exp op covers both, with
  exp work split ~50/50 between ScalarE and VectorE.
- FFN stays bf16 (fp8 there would blow the error budget) in two 512-wide
  groups; each group's FFN is a generator whose PE slices are drained one per
  attention unit, hiding exp latency under FFN matmuls. Residual stream is
  bf16 (hA: wo+residual -> LN1; h1b: LN1 out; hB: fc2+residual -> LN2) with
  separate tiles per role so group boundaries carry no WAR stalls.
- b_o + b_v@w_o is folded into the DMA'd residual (xtb = x + bo); b_q/b_k/b1/
  b2/be1 and g1 are folded into biases/weights on the host.
"""

import numpy as np
import ml_dtypes

import concourse.bass as bass
import concourse.tile as tile
from concourse import bacc
from concourse import mybir
from concourse.bass_utils import run_bass_kernel_spmd

P = 128
D = 1024          # d_model
S = 2048          # kv sequence length per core (one full batch)
TQ = 1024         # query tokens per core
H = 16            # heads
DK = 64           # head dim
FF = 4096         # ffn dim
DO = D // P       # 8  d_model chunks
KC = S // P       # 16 kv-token chunks
FO = FF // P      # 32 ffn chunks
NF = 512          # attention query-chunk width
NQA = TQ // NF    # 2 attention chunks
NG = 512          # ffn group width (2 attention chunks)
NW = 512          # projection free-dim tile
KH = KC // 2      # kv chunks per half
KP = KH // 2      # kv chunk-pairs per half

# fp8 power-of-2 scales
SX, SW, SQ, SV, SCTX = 16.0, 64.0, 2.0, 8.0, 32.0
KQ_SCALE = SQ / (SX * SW)        # 2^-9 on the QKV psum
V_SCALE = SV / (SX * SW)         # 2^-7
WO_SCALE = 1.0 / (SCTX * SW)     # 2^-11
EXP_SCALE = 1.0 / (8.0 * SQ * SQ)  # scores psum = 4*qk; exp arg = qk/8

# Schraudolph exp-to-e5m2: bits8 = round(s*A + B); exp(s*EXP_SCALE) ~ bitcast
SCH5_A = 4.0 * 1.4426950408889634 * EXP_SCALE   # 2^2 mantissa steps / octave
SCH5_B = 15.0 * 4.0 - 0.2316                    # exp-bias*4 - schraudolph corr

BF16 = mybir.dt.bfloat16
F32 = mybir.dt.float32
F32R = mybir.dt.float32r
F8E4 = mybir.dt.float8e4
F8E5 = mybir.dt.float8e5
I8 = mybir.dt.int8
AF = mybir.ActivationFunctionType
ALU = mybir.AluOpType
DR = mybir.MatmulPerfMode.DoubleRow


def build():
    nc = bacc.Bacc("TRN2", target_bir_lowering=False, debug=False, num_devices=8)

    xt8 = nc.dram_tensor("xt8", [P, 4, 2, S], F8E4, kind="ExternalInput").ap()
    wq8 = nc.dram_tensor("wq8", [P, 4, 2, D], F8E4, kind="ExternalInput").ap()
    wk8 = nc.dram_tensor("wk8", [P, 4, 2, D], F8E4, kind="ExternalInput").ap()
    wv8 = nc.dram_tensor("wv8", [P, 4, 2, D], F8E4, kind="ExternalInput").ap()
    wo8 = nc.dram_tensor("wo8", [P, 4, 2, D], F8E4, kind="ExternalInput").ap()
    xtb = nc.dram_tensor("xtb", [D, TQ], BF16, kind="ExternalInput").ap()
    w1s = nc.dram_tensor("w1s", [8, P, DO, 512], BF16, kind="ExternalInput").ap()
    w2s = nc.dram_tensor("w2s", [8, P, FO, P], BF16, kind="ExternalInput").ap()
    bq2 = nc.dram_tensor("bq2", [D], F32, kind="ExternalInput").ap()   # 2*b_q
    bk2 = nc.dram_tensor("bk2", [D], F32, kind="ExternalInput").ap()   # 2*b_k
    b1p = nc.dram_tensor("b1p", [FF], F32, kind="ExternalInput").ap()  # b1 + be1@w1
    b2e = nc.dram_tensor("b2e", [D], F32, kind="ExternalInput").ap()   # b2 + be1
    g1 = nc.dram_tensor("g1", [D], F32, kind="ExternalInput").ap()
    g2 = nc.dram_tensor("g2", [D], F32, kind="ExternalInput").ap()
    be2 = nc.dram_tensor("be2", [D], F32, kind="ExternalInput").ap()
    onesr = nc.dram_tensor("onesr", [P], BF16, kind="ExternalInput").ap()
    yt = nc.dram_tensor("yt", [D, TQ], F32R, kind="ExternalOutput").ap()

    xtb3 = xtb.rearrange("(o p) t -> p o t", p=P)
    yt3 = yt.rearrange("(o p) t -> p o t", p=P)

    with tile.TileContext(nc) as tc:
        with (
            tc.tile_pool(name="persist", bufs=1) as persist,
            tc.tile_pool(name="lns", bufs=1) as lns,
            tc.tile_pool(name="recp", bufs=2) as recp,
            tc.tile_pool(name="work", bufs=2) as work,
            tc.tile_pool(name="bcp", bufs=2) as bcp,
            tc.tile_pool(name="mm", bufs=2, space="PSUM") as mmp,
            tc.tile_pool(name="pssc", bufs=2, space="PSUM") as pssc,
            tc.tile_pool(name="pcp", bufs=2, space="PSUM") as pcp,
        ):
            def load_vec(ap, n_chunks, name):
                t = persist.tile([P, n_chunks], F32, tag=name)
                nc.sync.dma_start(out=t[:], in_=ap.rearrange("(o p) -> p o", p=P))
                return t

            eng_flip = [0]

            with tc.tile_pool(name="kqv", bufs=1) as kqvp:
                kT = kqvp.tile([P, DO, S], F8E4, tag="kT")
                qT = kqvp.tile([P, DO, TQ], F8E4, tag="qT")
                vaug = kqvp.tile([P, KC // 2, 2, H * 65], F8E4, tag="vaug")
                vaug_h = vaug.rearrange("p g l (h w) -> p g l h w", w=65)

                # ---------------- phase 1: QKV projections (fp8 DoubleRow) ----
                with (
                    tc.tile_pool(name="xtp", bufs=1) as xtp,
                    tc.tile_pool(name="wst", bufs=2) as wst,
                ):
                    wk_sb = wst.tile([P, 4, 2, D], F8E4, tag="w", name="w_k")
                    nc.sync.dma_start(out=wk_sb[:], in_=wk8)
                    xt_sb = xtp.tile([P, 4, 2, S], F8E4, tag="xt")
                    for n in range(S // NW):
                        nc.sync.dma_start(out=xt_sb[:, :, :, bass.ts(n, NW)],
                                          in_=xt8[:, :, :, bass.ts(n, NW)])
                    bq_sb = load_vec(bq2, DO, "bq")
                    bk_sb = load_vec(bk2, DO, "bk")
                    b2e_sb = load_vec(b2e, DO, "b2e")
                    g1_sb = load_vec(g1, DO, "g1")
                    g2_sb = load_vec(g2, DO, "g2")
                    be2_sb = load_vec(be2, DO, "be2")
                    b1p_sb = load_vec(b1p, FO, "b1p")
                    ones_col = persist.tile([P, 1], BF16, tag="ones_col")
                    nc.sync.dma_start(out=ones_col[:], in_=onesr[:, None])
                    for t in range(KC):
                        nc.vector.memset(vaug_h[:, t // 2, t % 2, :, 64:65], 0.25)

                    def proj_epilogue(out_ap, ps, bias_col, scale):
                        # alternate ACT/DVE so both engines share QKV epilogues
                        eng_flip[0] ^= 1
                        if eng_flip[0]:
                            if bias_col is None:
                                nc.scalar.activation(out=out_ap, in_=ps,
                                                     func=AF.Identity, scale=scale)
                            else:
                                nc.scalar.activation(out=out_ap, in_=ps,
                                                     func=AF.Identity,
                                                     bias=bias_col, scale=scale)
                        else:
                            if bias_col is None:
                                nc.vector.tensor_scalar(
                                    out=out_ap, in0=ps, scalar1=scale,
                                    scalar2=None, op0=ALU.mult)
                            else:
                                nc.vector.tensor_scalar(
                                    out=out_ap, in0=ps, scalar1=scale,
                                    scalar2=bias_col, op0=ALU.mult, op1=ALU.add)

                    def kq_slice(w_sb, out_sb, bias_sb, n, m, nm):
                        ps = mmp.tile([P, NW], F32, tag="mm", name=f"mm_{nm}")
                        for kp in range(4):
                            nc.tensor.matmul(
                                ps[:], lhsT=w_sb[:, kp, :, bass.ts(m, P)],
                                rhs=xt_sb[:, kp, :, bass.ts(n, NW)],
                                start=(kp == 0), stop=(kp == 3),
                                perf_mode=DR)
                        proj_epilogue(out_sb[:, m, bass.ts(n, NW)], ps[:],
                                      bias_sb[:, m:m + 1], KQ_SCALE)

                    for n in range(S // NW):
                        for m in range(DO):
                            kq_slice(wk_sb, kT, bk_sb, n, m, f"k{n}_{m}")
                    wq_sb = wst.tile([P, 4, 2, D], F8E4, tag="w", name="w_q")
                    nc.sync.dma_start(out=wq_sb[:], in_=wq8)
                    for n in range(TQ // NW):
                        for m in range(DO):
                            kq_slice(wq_sb, qT, bq_sb, n, m, f"q{n}_{m}")

                    # V in natural [token, d] layout, heads padded to 65 cols
                    wv_sb = wst.tile([P, 4, 2, D], F8E4, tag="w", name="w_v")
                    nc.sync.dma_start(out=wv_sb[:], in_=wv8)
                    for t in range(KC):
                        for n in range(2):
                            ps = mmp.tile([P, NW], F32, tag="mm",
                                          name=f"mmv_{t}_{n}")
                            for kp in range(4):
                                nc.tensor.matmul(
                                    ps[:], lhsT=xt_sb[:, kp, :, bass.ts(t, P)],
                                    rhs=wv_sb[:, kp, :, bass.ts(n, NW)],
                                    start=(kp == 0), stop=(kp == 3),
                                    perf_mode=DR)
                            psh = ps.rearrange("p (h w) -> p h w", w=DK)
                            dst = vaug_h[:, t // 2, t % 2, 8 * n:8 * n + 8, 0:64]
                            proj_epilogue(dst, psh, None, V_SCALE)

                # ------------- phase 2: attention + interleaved FFN -----------
                with (
                    tc.tile_pool(name="es", bufs=2) as esp,
                    tc.tile_pool(name="ctxp", bufs=2) as ctxp,
                    tc.tile_pool(name="wop", bufs=1) as wop,
                    tc.tile_pool(name="w12", bufs=2) as w12,
                    tc.tile_pool(name="ffs2", bufs=1) as ffs2,
                    tc.tile_pool(name="ffh1", bufs=1) as ffh1,
                    tc.tile_pool(name="ffhb", bufs=1) as ffhb,
                    tc.tile_pool(name="ffa", bufs=1) as ffa,
                ):
                    wo_sb = wop.tile([P, 4, 2, D], F8E4, tag="wo")
                    nc.sync.dma_start(out=wo_sb[:], in_=wo8)

                    exp_acc = [0.0]
                    ACT_FRAC = 0.62

                    def emit_exp(dst_f8, ps):
                        """softmax exp of one [P,2,NF] scores tile -> e5m2."""
                        exp_acc[0] += ACT_FRAC
                        if exp_acc[0] >= 1.0:
                            exp_acc[0] -= 1.0
                            nc.scalar.activation(out=dst_f8, in_=ps[:],
                                                 func=AF.Exp, scale=EXP_SCALE)
                        else:
                            nc.vector.tensor_scalar(
                                out=dst_f8.bitcast(I8), in0=ps[:],
                                scalar1=SCH5_A, scalar2=SCH5_B,
                                op0=ALU.mult, op1=ALU.add)

                    def emit_scores(qn, j, half):
                        """scores+exp for 8 kv chunks x both heads of pair j.
                        es layout [P, idx, KH, NF]; psum tiles pack idx0/idx1
                        as free-dim planes so one [P,1024] exp covers both."""
                        qsl = bass.ts(qn, NF)
                        es_u = esp.tile([P, 2, KH, NF], F8E5, tag="es",
                                        name=f"es_{qn}_{j}_{half}")
                        for kl in range(KH):
                            kc = half * KH + kl
                            pss = pssc.tile([P, 2, NF], F32, tag="sc",
                                            name=f"sc_{kl % 2}")
                            for idx in range(2):
                                off = idx * DK
                                nc.tensor.matmul(
                                    pss[:, idx, :],
                                    lhsT=kT[off:off + DK, j, bass.ts(kc, P)],
                                    rhs=qT[off:off + DK, j, qsl],
                                    start=True, stop=True)
                            emit_exp(es_u[:, :, kl, :], pss)
                        return es_u

                    def emit_av(qn, j, half, es_u, pcs, ctx_t):
                        for idx in range(2):
                            h = 2 * j + idx
                            pc = pcs[idx]
                            for kp in range(KP):
                                pg = half * KP + kp
                                nc.tensor.matmul(
                                    pc[0:DK + 1, :],
                                    lhsT=vaug[:, pg, :, h * 65:(h + 1) * 65],
                                    rhs=es_u[:, idx, 2 * kp:2 * kp + 2, :],
                                    start=(pg == 0), stop=(pg == 2 * KP - 1),
                                    perf_mode=DR)
                            if half == 0:
                                continue
                            den = lns.tile([1, NF], F32, tag="den",
                                           name=f"den_{qn}_{h}")
                            nc.vector.tensor_copy(den[:], pc[DK:DK + 1, :])
                            rec = recp.tile([1, NF], F32, tag="rec",
                                            name=f"rec_{qn}_{h}")
                            nc.vector.reciprocal_approx_fast(
                                out=rec[:], in_=den[:])
                            recb = bcp.tile([DK, NF], F32, tag="recb",
                                            name=f"recb_{qn}_{h}")
                            nc.gpsimd.partition_broadcast(recb[:], rec[:],
                                                          channels=DK)
                            nc.vector.tensor_mul(
                                ctx_t[idx * DK:(idx + 1) * DK, j, :],
                                pc[0:DK, :], recb[:])

                    # ---- FFN for one 512-wide group, as PE-slice generator ----
                    def ln_stats(h3, tagp):
                        """LN over the partition (feature) dim of h3 [P,DO,NG].
                        sum in psum row 0, sumsq in row 32 (col-packed).
                        Yields once mid-way; returns (rstd_b, negms_b)."""
                        ps_s = mmp.tile([P, NW], F32, tag="mm", name=f"{tagp}_s")
                        ps_q = mmp.tile([P, NW], F32, tag="mm", name=f"{tagp}_q")
                        for o in range(DO):
                            nc.tensor.matmul(
                                ps_s[0:1, 0:NG], lhsT=ones_col,
                                rhs=h3[:, o, :],
                                start=(o == 0), stop=(o == DO - 1))
                            sq = work.tile([P, NG], BF16, tag="sq",
                                           name=f"{tagp}_sq{o % 2}")
                            nc.scalar.activation(out=sq[:], in_=h3[:, o, :],
                                                 func=AF.Square)
                            nc.tensor.matmul(
                                ps_q[0:1, 0:NG], lhsT=ones_col, rhs=sq[:],
                                start=(o == 0), stop=(o == DO - 1))
                            if o == 3:
                                yield None
                        mean_n = lns.tile([1, NG], F32, tag="mn", name=f"{tagp}_mn")
                        nc.vector.tensor_scalar(
                            out=mean_n[:], in0=ps_s[0:1, 0:NG], scalar1=-1.0 / D,
                            scalar2=None, op0=ALU.mult)
                        msq_e = lns.tile([1, NG], F32, tag="me", name=f"{tagp}_me")
                        nc.vector.tensor_scalar(
                            out=msq_e[:], in0=ps_q[0:1, 0:NG], scalar1=1.0 / D,
                            scalar2=1e-5, op0=ALU.mult, op1=ALU.add)
                        m2 = lns.tile([1, NG], F32, tag="m2", name=f"{tagp}_m2")
                        nc.vector.tensor_mul(m2[:], mean_n[:], mean_n[:])
                        var = lns.tile([1, NG], F32, tag="va", name=f"{tagp}_va")
                        nc.vector.scalar_tensor_tensor(
                            out=var[:], in0=m2[:], scalar=-1.0, in1=msq_e[:],
                            op0=ALU.mult, op1=ALU.add)
                        lv = lns.tile([1, NG], F32, tag="lv", name=f"{tagp}_lv")
                        nc.scalar.activation(out=lv[:], in_=var[:], func=AF.Ln)
                        rstd = lns.tile([1, NG], F32, tag="rs", name=f"{tagp}_rs")
                        nc.scalar.activation(out=rstd[:], in_=lv[:],
                                             func=AF.Exp, scale=-0.5)
                        negms = lns.tile([1, NG], F32, tag="ng", name=f"{tagp}_ng")
                        nc.vector.tensor_mul(negms[:], mean_n[:], rstd[:])
                        rstd_b = bcp.tile([P, NG], F32, tag="lnb",
                                          name=f"{tagp}_rb")
                        nc.gpsimd.partition_broadcast(rstd_b[:], rstd[:])
                        negms_b = bcp.tile([P, NG], F32, tag="lnb",
                                           name=f"{tagp}_nb")
                        nc.gpsimd.partition_broadcast(negms_b[:], negms[:])
                        yield (rstd_b, negms_b)

                    def ffn_gen(g, ctx_t):
                        sl = bass.ts(g, NG)
                        xq_sb = ffs2.tile([P, DO, NG], BF16, tag="xq",
                                          name=f"xq_{g}")
                        nc.sync.dma_start(out=xq_sb[:], in_=xtb3[:, :, sl])
                        w1_t = [None, None]
                        w2_t = [None, None]

                        def load_w1(c):
                            w1_t[c % 2] = w12.tile([P, DO, 512], BF16, tag="w1",
                                                   name=f"w1_{g}_{c}")
                            nc.gpsimd.dma_start(out=w1_t[c % 2][:], in_=w1s[c])

                        def load_w2(m):
                            w2_t[m % 2] = w12.tile([P, FO, P], BF16, tag="w2",
                                                   name=f"w2_{g}_{m}")
                            nc.gpsimd.dma_start(out=w2_t[m % 2][:], in_=w2s[m])

                        load_w1(0)
                        load_w1(1)
                        hA = ffh1.tile([P, DO, NG], BF16, tag="hA",
                                       name=f"hA_{g}")
                        hB = ffh1.tile([P, DO, NG], BF16, tag="hB",
                                       name=f"hB_{g}")
                        # w_o (fp8 DoubleRow) + residual (bo pre-folded into xtb)
                        for m in range(DO):
                            ps = mmp.tile([P, NW], F32, tag="mm",
                                          name=f"wo_{g}_{m % 2}")
                            for kp in range(4):
                                nc.tensor.matmul(
                                    ps[:, 0:NG], lhsT=wo_sb[:, kp, :, bass.ts(m, P)],
                                    rhs=ctx_t[:, 2 * kp:2 * kp + 2, :],
                                    start=(kp == 0), stop=(kp == 3),
                                    perf_mode=DR)
                            nc.vector.scalar_tensor_tensor(
                                out=hA[:, m, :], in0=ps[:, 0:NG],
                                scalar=WO_SCALE, in1=xq_sb[:, m, :],
                                op0=ALU.mult, op1=ALU.add)
                            if m % 2 == 1:
                                yield
                        # LN1 (no affine; g1/be1 folded into w1/b1p/fc2 epilogue)
                        gst = ln_stats(hA, f"l1_{g}")
                        next(gst)
                        yield
                        rstd_b, negms_b = next(gst)
                        h1b = ffhb.tile([P, DO, NG], BF16, tag="h1b",
                                        name=f"h1b_{g}")
                        for o in range(DO):
                            nc.vector.tensor_mul(hA[:, o, :], hA[:, o, :],
                                                 rstd_b[:])
                            nc.vector.tensor_add(h1b[:, o, :], hA[:, o, :],
                                                 negms_b[:])
                            if o % 4 == 3:
                                yield
                        # fc1: aT = relu(w1g^T @ h1n + b1p)   (bf16)
                        aT = ffa.tile([P, FO, NG], BF16, tag="aT",
                                      name=f"aT_{g}")
                        for c in range(8):
                            w1c = w1_t[c % 2]
                            for u in range(4):
                                mf = 4 * c + u
                                ps = mmp.tile([P, NW], F32, tag="mm",
                                              name=f"f1_{g}_{mf % 2}")
                                for kc in range(DO):
                                    nc.tensor.matmul(
                                        ps[:, 0:NG],
                                        lhsT=w1c[:, kc, bass.ts(u, P)],
                                        rhs=h1b[:, kc, :],
                                        start=(kc == 0), stop=(kc == DO - 1))
                                nc.scalar.activation(
                                    out=aT[:, mf, :], in_=ps[:, 0:NG],
                                    func=AF.Relu, bias=b1p_sb[:, mf:mf + 1])
                            if c + 2 < 8:
                                load_w1(c + 2)
                            elif c == 6:
                                load_w2(0)
                            else:
                                load_w2(1)
                            yield
                        # fc2 + residual (h2 = h1n*g1 + be1 + ff + b2)
                        for m in range(DO):
                            w2c = w2_t[m % 2]
                            ps = mmp.tile([P, NW], F32, tag="mm",
                                          name=f"f2_{g}_{m % 2}")
                            for kc in range(FO):
                                nc.tensor.matmul(
                                    ps[:, 0:NG], lhsT=w2c[:, kc, :],
                                    rhs=aT[:, kc, :],
                                    start=(kc == 0), stop=(kc == FO - 1))
                            ep = work.tile([P, NG], F32, tag="ep",
                                           name=f"ep_{g}_{m % 2}")
                            nc.scalar.activation(out=ep[:], in_=ps[:, 0:NG],
                                                 func=AF.Identity,
                                                 bias=b2e_sb[:, m:m + 1])
                            nc.vector.scalar_tensor_tensor(
                                out=hB[:, m, :], in0=h1b[:, m, :],
                                scalar=g1_sb[:, m:m + 1], in1=ep[:],
                                op0=ALU.mult, op1=ALU.add)
                            if m + 2 < DO:
                                load_w2(m + 2)
                            yield
                        # LN2 -> yt
                        gst = ln_stats(hB, f"l2_{g}")
                        next(gst)
                        yield
                        rstd_b, negms_b = next(gst)
                        # write LN2 into scratch tiles (hB frees at the muls)
                        for o in range(DO):
                            tmp = work.tile([P, NG], F32R, tag="ep",
                                            name=f"yt_{g}_{o % 2}")
                            nc.vector.tensor_mul(tmp[:], hB[:, o, :],
                                                 rstd_b[:])
                            nc.vector.tensor_add(tmp[:], tmp[:], negms_b[:])
                            nc.scalar.activation(
                                out=tmp[:], in_=tmp[:], func=AF.Identity,
                                scale=g2_sb[:, o:o + 1],
                                bias=be2_sb[:, o:o + 1])
                            nc.sync.dma_start(out=yt3[:, o, sl],
                                              in_=tmp[:])
                            if o % 4 == 3:
                                yield

                    # ---- main interleaved emission ----
                    units = [(qn, j, half) for qn in range(NQA)
                             for j in range(H // 2) for half in range(2)]
                    gens = []

                    def drain(n):
                        k = 0
                        while k < n and gens:
                            try:
                                next(gens[0])
                                k += 1
                            except StopIteration:
                                gens.pop(0)

                    ctx_tiles = {}
                    pcs = None
                    prev = emit_scores(*units[0])
                    for i, (qn, j, half) in enumerate(units):
                        if (j, half) == (0, 0):
                            ctx_tiles[qn] = ctxp.tile(
                                [P, DO, NG], F8E4, tag="ctx",
                                name=f"ctx_{qn}")
                        cur = prev
                        if i + 1 < len(units):
                            prev = emit_scores(*units[i + 1])
                        if half == 0:
                            pcs = [pcp.tile([P, NF], F32, tag="pc",
                                            name=f"pc_{qn}_{j}_{i2}")
                                   for i2 in range(2)]
                        drain(2 if i >= 24 else 1)
                        emit_av(qn, j, half, cur, pcs, ctx_tiles[qn])
                        if (j, half) == (H // 2 - 1, 1):
                            gens.append(ffn_gen(qn, ctx_tiles[qn]))
                    drain(10 ** 6)

    nc.compile()
    return nc


_CACHE = {}


def _compiled():
    if "nc" not in _CACHE:
        _CACHE["nc"] = build()
    return _CACHE["nc"]


def make_in_maps(x, w_q, b_q, w_k, b_k, w_v, b_v, w_o, b_o,
                 w1, b1, w2, b2, g1, be1, g2, be2):
    bf = ml_dtypes.bfloat16
    f8 = ml_dtypes.float8_e4m3
    x = np.asarray(x, np.float32)
    f32 = lambda a: np.ascontiguousarray(np.asarray(a, np.float32))

    def w_dr(w, scale):
        # [D, M] -> [P, 4, 2, M] fp8 with contraction chunk pairs interleaved
        a = np.clip(f32(w) * scale, -240.0, 240.0)
        return np.ascontiguousarray(
            a.reshape(4, 2, P, -1).transpose(2, 0, 1, 3)).astype(f8)

    w_o32 = f32(w_o)
    w1_32 = f32(w1)
    g1_32 = f32(g1)
    be1_32 = f32(be1)
    w1g = w1_32 * g1_32[:, None]
    w1r = np.ascontiguousarray(
        w1g.reshape(DO, P, FF).transpose(1, 0, 2))          # [P, DO, FF]
    w1s_h = np.ascontiguousarray(
        w1r.reshape(P, DO, 8, 512).transpose(2, 0, 1, 3)).astype(bf)
    w2_32 = f32(w2)
    w2r = np.ascontiguousarray(
        w2_32.reshape(FO, P, D).transpose(1, 0, 2))          # [P, FO, D]
    w2s_h = np.ascontiguousarray(
        w2r.reshape(P, FO, DO, P).transpose(2, 0, 1, 3)).astype(bf)

    bo = f32(b_o) + f32(b_v) @ w_o32
    shared = {
        "wq8": w_dr(w_q, SW), "wk8": w_dr(w_k, SW),
        "wv8": w_dr(w_v, SW), "wo8": w_dr(w_o, SW),
        "w1s": w1s_h, "w2s": w2s_h,
        "bq2": f32(b_q) * SQ, "bk2": f32(b_k) * SQ,
        "b1p": f32(b1) + be1_32 @ w1_32,
        "b2e": f32(b2) + be1_32,
        "g1": g1_32, "g2": f32(g2), "be2": f32(be2),
        "onesr": np.ones((P,), ml_dtypes.bfloat16),
    }
    in_maps = []
    for c in range(8):
        b, r = c // 2, c % 2
        xb = x[b]
        xc = np.concatenate([xb[r * TQ:(r + 1) * TQ],
                             xb[(1 - r) * TQ:(2 - r) * TQ]], axis=0)
        m = dict(shared)
        xcT = np.ascontiguousarray(xc.T)                     # [D, S]
        m["xt8"] = np.ascontiguousarray(
            np.clip(xcT * SX, -240.0, 240.0)
            .reshape(4, 2, P, S).transpose(2, 0, 1, 3)).astype(f8)
        m["xtb"] = np.ascontiguousarray(
            (xb[r * TQ:(r + 1) * TQ] + bo[None, :]).T).astype(bf)
        in_maps.append(m)
    return in_maps


def assemble_out(results):
    out = np.empty((4, 2048, 1024), np.float32)
    for c in range(8):
        b, r = c // 2, c % 2
        out[b, r * TQ:(r + 1) * TQ] = results[c]["yt"].T
    return out


def kernel(x, src_mask, w_q, b_q, w_k, b_k, w_v, b_v, w_o, b_o,
           w1, b1, w2, b2, g1, be1, g2, be2):
    in_maps = make_in_maps(x, w_q, b_q, w_k, b_k, w_v, b_v, w_o, b_o,
                           w1, b1, w2, b2, g1, be1, g2, be2)
    nc = _compiled()
    res = run_bass_kernel_spmd(nc, in_maps, core_ids=list(range(8)))
    return assemble_out(res.results)
